# revision 4
# baseline (speedup 1.0000x reference)
"""Trainium2 Bass kernel for segment_sum (scatter-add of edge features into nodes).

Strategy: 2M edges split contiguously across 8 NeuronCores (250k each).
Host-side prep (layout only, no FP arithmetic): sort each core's edges by
node id, cut the sorted stream into 128 partition streams at node-run
boundaries, pad each stream to 2048 slots, and build a run-continuation
mask m (m=0 at the first edge of each node run, 1 inside a run).

Device (per core): the whole reduction is a segmented scan on the DVE:
    state = m[t] * state + h[t]        (fp32 internal state)
run per feature channel d (32 contiguous scans per 512-slot piece,
chained across pieces via `initial`). At the last slot of each node run,
`state` holds that node's complete per-core sum. The full scan stream is
DMA'd back (bf16) and the host picks the run-end slots and adds the 8
per-core partials (same unshard-add as the original baseline).

No PE, no GPSIMD, no gather: HBM traffic is 2 x 16.8 MB/core of
contiguous bf16 and the DVE scan runs at ~2 cycles/element.
"""
import numpy as np
import ml_dtypes

import concourse.bacc as bacc
import concourse.mybir as mybir
from concourse import tile
from concourse.bass_utils import run_bass_kernel_spmd

BF16 = mybir.dt.bfloat16
OP = mybir.AluOpType

E = 2_000_000
D = 32
N = 100_000
CORES = 8
EPC = E // CORES            # 250_000
PARTS = 128
SLOTS = 2016                # padded edge slots per partition stream
PIECES = 2
PLEN = SLOTS // PIECES      # 1008 slots per piece
PFREE = D * PLEN            # free elements per piece
FREE = PIECES * PFREE
# input d-groups (first group small so the first scan starts early)
IN_GROUPS = [2, 2, 4, 4, 4, 4, 4, 4, 4]
# output d-groups per piece (last group of the last piece split fine to
# shrink the final-DMA tail)
OUT_GROUPS = [4, 4, 4, 4, 4, 4, 4, 2, 1, 1]


def build_program():
    nc = bacc.Bacc("TRN2", target_bir_lowering=False, debug=False,
                   num_devices=CORES)
    h_in = nc.dram_tensor("h", [PARTS, FREE], BF16, kind="ExternalInput")
    m_in = nc.dram_tensor("m", [PARTS, SLOTS], BF16, kind="ExternalInput")
    s_out = nc.dram_tensor("s", [PARTS, FREE], BF16, kind="ExternalOutput")

    with tile.TileContext(nc) as tc:
        with tc.tile_pool(name="mask", bufs=1) as mp, \
             tc.tile_pool(name="work", bufs=2) as wp:
            mt = mp.tile([PARTS, SLOTS], BF16)
            nc.sync.dma_start(mt[:], m_in[:])
            tiles = []
            # issue every input DMA upfront; both piece buffers exist
            # (bufs=2) so piece 1 streams in while piece 0 scans
            for k in range(PIECES):
                ht = wp.tile([PARTS, PFREE], BF16, tag="h")
                d0 = 0
                for ng in IN_GROUPS:
                    lo = d0 * PLEN
                    hi = (d0 + ng) * PLEN
                    nc.sync.dma_start(
                        ht[:, lo:hi],
                        h_in[:, k * PFREE + lo:k * PFREE + hi])
                    d0 += ng
                tiles.append(ht)
            prev = None
            for k in range(PIECES):
                ht = tiles[k]
                d = 0
                for ng in OUT_GROUPS:
                    for dd in range(d, d + ng):
                        lo = dd * PLEN
                        hi = lo + PLEN
                        init = 0.0 if prev is None else prev[:, hi - 1:hi]
                        # in-place: the scan overwrites the h tile
                        nc.vector.tensor_tensor_scan(
                            ht[:, lo:hi],
                            mt[:, k * PLEN:(k + 1) * PLEN],
                            ht[:, lo:hi],
                            init, OP.mult, OP.add)
                    lo = d * PLEN
                    hi = (d + ng) * PLEN
                    nc.sync.dma_start(
                        s_out[:, k * PFREE + lo:k * PFREE + hi],
                        ht[:, lo:hi])
                    d += ng
                prev = ht
    nc.compile()
    return nc


_prog_cache = {}


def _get_prog():
    if "nc" not in _prog_cache:
        _prog_cache["nc"] = build_program()
    return _prog_cache["nc"]


def kernel(H, X_node, node_num):
    H = np.ascontiguousarray(np.asarray(H, dtype=np.float32))
    X = np.asarray(X_node).astype(np.int64)
    assert H.shape == (E, D) and X.shape == (E,)
    nc = _get_prog()

    in_maps = []
    metas = []
    tgt = np.arange(1, PARTS) * ((EPC + PARTS - 1) // PARTS)
    for c in range(CORES):
        Xc = X[c * EPC:(c + 1) * EPC]
        Hc = H[c * EPC:(c + 1) * EPC]
        perm = np.argsort(Xc, kind="stable")
        Xs = Xc[perm]
        Hs = Hc[perm]
        # node-run starts; cut the stream into 128 partition streams at
        # run boundaries so no node spans two partitions
        runstarts = np.concatenate(
            [[0], np.flatnonzero(np.diff(Xs)) + 1])
        ci = np.searchsorted(runstarts, tgt, side="left")
        ci = np.minimum(ci, len(runstarts) - 1)
        cuts = np.concatenate([[0], runstarts[ci], [EPC]])
        cnt = np.diff(cuts)
        assert cnt.max() <= SLOTS, f"partition stream overflow: {cnt.max()}"

        node_pad = np.full((PARTS, SLOTS), -1, np.int64)
        h_pad = np.zeros((PARTS, SLOTS, D), np.float32)
        pidx = np.repeat(np.arange(PARTS), cnt)
        eidx = np.arange(EPC) - np.repeat(cuts[:-1], cnt)
        node_pad[pidx, eidx] = Xs
        h_pad[pidx, eidx] = Hs
        m = np.zeros((PARTS, SLOTS), np.float32)
        m[:, 1:] = node_pad[:, 1:] == node_pad[:, :-1]

        h_dev = np.ascontiguousarray(
            h_pad.reshape(PARTS, PIECES, PLEN, D).transpose(0, 1, 3, 2)
        ).reshape(PARTS, FREE).astype(ml_dtypes.bfloat16)
        m_dev = m.astype(ml_dtypes.bfloat16)
        in_maps.append({"h": h_dev, "m": np.ascontiguousarray(m_dev)})
        metas.append(node_pad)

    _prog_cache["last_inputs"] = in_maps
    res = run_bass_kernel_spmd(nc, in_maps, core_ids=list(range(CORES)),
                               trace=False)

    out = np.zeros((N, D), np.float32)
    for c in range(CORES):
        node_pad = metas[c]
        s = np.asarray(res.results[c]["s"]).astype(np.float32)
        s = s.reshape(PARTS, PIECES, D, PLEN)
        nxt = np.concatenate(
            [node_pad[:, 1:], np.full((PARTS, 1), -2, np.int64)], axis=1)
        is_end = (node_pad >= 0) & (node_pad != nxt)
        pp, ii = np.nonzero(is_end)
        nodes = node_pad[pp, ii]
        vals = s[pp, ii // PLEN, :, ii % PLEN]
        # within one core each node has exactly one run end -> unique idx
        out[nodes] += vals
    return out


# revision 6
# speedup vs baseline: 1.1774x; 1.1774x over previous
"""Trainium2 Bass kernel for segment_sum (scatter-add of edge features into nodes).

Strategy: 2M edges split contiguously across 8 NeuronCores (250k each).
Host-side prep (layout only, no FP arithmetic): sort each core's edges by
node id, cut the sorted stream into 128 partition streams at node-run
boundaries, pad each stream to 2048 slots, and build a run-continuation
mask m (m=0 at the first edge of each node run, 1 inside a run).

Device (per core): the whole reduction is a segmented scan on the DVE:
    state = m[t] * state + h[t]        (fp32 internal state)
run per feature channel d (32 contiguous scans per 512-slot piece,
chained across pieces via `initial`). At the last slot of each node run,
`state` holds that node's complete per-core sum. The full scan stream is
DMA'd back (bf16) and the host picks the run-end slots and adds the 8
per-core partials (same unshard-add as the original baseline).

No PE, no GPSIMD, no gather: HBM traffic is 2 x 16.8 MB/core of
contiguous bf16 and the DVE scan runs at ~2 cycles/element.
"""
import numpy as np
import ml_dtypes

import concourse.bacc as bacc
import concourse.mybir as mybir
from concourse import tile
from concourse.bass_utils import run_bass_kernel_spmd

BF16 = mybir.dt.bfloat16
OP = mybir.AluOpType

E = 2_000_000
D = 32
N = 100_000
CORES = 8
EPC = E // CORES            # 250_000
PARTS = 128
SLOTS = 2048                # padded edge slots per partition stream
PIECES = 2
PLEN = SLOTS // PIECES      # 1024 slots per piece
PFREE = D * PLEN            # free elements per piece
FREE = PIECES * PFREE
# input d-groups (first group small so the first scan starts early)
IN_GROUPS = [2, 2, 4, 4, 4, 4, 4, 4, 4]
# output d-groups per piece (last group of the last piece split fine to
# shrink the final-DMA tail)
OUT_GROUPS = [4, 4, 4, 4, 4, 4, 4, 2, 1, 1]


def build_program():
    nc = bacc.Bacc("TRN2", target_bir_lowering=False, debug=False,
                   num_devices=CORES)
    h_in = nc.dram_tensor("h", [PARTS, FREE], BF16, kind="ExternalInput")
    m_in = nc.dram_tensor("m", [PARTS, SLOTS], BF16, kind="ExternalInput")
    s_out = nc.dram_tensor("s", [PARTS, FREE], BF16, kind="ExternalOutput")

    with tile.TileContext(nc) as tc:
        with tc.tile_pool(name="mask", bufs=1) as mp, \
             tc.tile_pool(name="work", bufs=2) as wp:
            mt = mp.tile([PARTS, SLOTS], BF16)
            nc.sync.dma_start(mt[:], m_in[:])
            tiles = []
            # issue every input DMA upfront; both piece buffers exist
            # (bufs=2) so piece 1 streams in while piece 0 scans
            for k in range(PIECES):
                ht = wp.tile([PARTS, PFREE], BF16, tag="h")
                d0 = 0
                for ng in IN_GROUPS:
                    lo = d0 * PLEN
                    hi = (d0 + ng) * PLEN
                    nc.sync.dma_start(
                        ht[:, lo:hi],
                        h_in[:, k * PFREE + lo:k * PFREE + hi])
                    d0 += ng
                tiles.append(ht)
            prev = None
            for k in range(PIECES):
                ht = tiles[k]
                d = 0
                for ng in OUT_GROUPS:
                    for dd in range(d, d + ng):
                        lo = dd * PLEN
                        hi = lo + PLEN
                        init = 0.0 if prev is None else prev[:, hi - 1:hi]
                        # in-place: the scan overwrites the h tile
                        nc.vector.tensor_tensor_scan(
                            ht[:, lo:hi],
                            mt[:, k * PLEN:(k + 1) * PLEN],
                            ht[:, lo:hi],
                            init, OP.mult, OP.add)
                    lo = d * PLEN
                    hi = (d + ng) * PLEN
                    nc.sync.dma_start(
                        s_out[:, k * PFREE + lo:k * PFREE + hi],
                        ht[:, lo:hi])
                    d += ng
                prev = ht
    nc.compile()
    return nc


_prog_cache = {}


def _get_prog():
    if "nc" not in _prog_cache:
        _prog_cache["nc"] = build_program()
    return _prog_cache["nc"]


def kernel(H, X_node, node_num):
    H = np.ascontiguousarray(np.asarray(H, dtype=np.float32))
    X = np.asarray(X_node).astype(np.int64)
    assert H.shape == (E, D) and X.shape == (E,)
    nc = _get_prog()

    in_maps = []
    metas = []
    tgt = np.arange(1, PARTS) * ((EPC + PARTS - 1) // PARTS)
    for c in range(CORES):
        Xc = X[c * EPC:(c + 1) * EPC]
        Hc = H[c * EPC:(c + 1) * EPC]
        perm = np.argsort(Xc, kind="stable")
        Xs = Xc[perm]
        Hs = Hc[perm]
        # node-run starts; cut the stream into 128 partition streams at
        # run boundaries so no node spans two partitions
        runstarts = np.concatenate(
            [[0], np.flatnonzero(np.diff(Xs)) + 1])
        ci = np.searchsorted(runstarts, tgt, side="left")
        ci = np.minimum(ci, len(runstarts) - 1)
        cuts = np.concatenate([[0], runstarts[ci], [EPC]])
        cnt = np.diff(cuts)
        assert cnt.max() <= SLOTS, f"partition stream overflow: {cnt.max()}"

        node_pad = np.full((PARTS, SLOTS), -1, np.int64)
        h_pad = np.zeros((PARTS, SLOTS, D), np.float32)
        pidx = np.repeat(np.arange(PARTS), cnt)
        eidx = np.arange(EPC) - np.repeat(cuts[:-1], cnt)
        node_pad[pidx, eidx] = Xs
        h_pad[pidx, eidx] = Hs
        m = np.zeros((PARTS, SLOTS), np.float32)
        m[:, 1:] = node_pad[:, 1:] == node_pad[:, :-1]

        h_dev = np.ascontiguousarray(
            h_pad.reshape(PARTS, PIECES, PLEN, D).transpose(0, 1, 3, 2)
        ).reshape(PARTS, FREE).astype(ml_dtypes.bfloat16)
        m_dev = m.astype(ml_dtypes.bfloat16)
        in_maps.append({"h": h_dev, "m": np.ascontiguousarray(m_dev)})
        metas.append(node_pad)

    _prog_cache["last_inputs"] = in_maps
    # The very first execution of a freshly loaded program has been
    # observed (once) to return corrupted results; correct runs are
    # bit-identical. Run until two consecutive executions agree.
    res = run_bass_kernel_spmd(nc, in_maps, core_ids=list(range(CORES)),
                               trace=False)
    for _ in range(3):
        res2 = run_bass_kernel_spmd(nc, in_maps, core_ids=list(range(CORES)),
                                    trace=False)
        if all(
            np.array_equal(
                res.results[c]["s"].view(np.uint16),
                res2.results[c]["s"].view(np.uint16))
            for c in range(CORES)
        ):
            break
        res = res2

    out = np.zeros((N, D), np.float32)
    for c in range(CORES):
        node_pad = metas[c]
        s = np.asarray(res.results[c]["s"]).astype(np.float32)
        s = s.reshape(PARTS, PIECES, D, PLEN)
        nxt = np.concatenate(
            [node_pad[:, 1:], np.full((PARTS, 1), -2, np.int64)], axis=1)
        is_end = (node_pad >= 0) & (node_pad != nxt)
        pp, ii = np.nonzero(is_end)
        nodes = node_pad[pp, ii]
        vals = s[pp, ii // PLEN, :, ii % PLEN]
        # within one core each node has exactly one run end -> unique idx
        out[nodes] += vals
    return out
